# revision 3
# baseline (speedup 1.0000x reference)
"""ContrastiveMagnitudeLoss on 8 Trainium2 NeuronCores (Bass/Tile).

Strategy (sharding_hint: shard batch across cores, all-gather target):
  - B=4096 rows of `predicted` are sharded 512/core. Every core gets the
    full (transposed) `target`, so each core owns complete rows of the
    B x B distance matrix and the row-softmax needs no communication.
  - The Gram identity  d^2[m,n] = ||p_m||^2 + ||t_n||^2 - 2 p_m.t_n  is
    computed entirely on the PE array by extending the contraction dim:
    4 extra K-rows carry (1, -tsq/2) and (-psq/2, 1) rank-1 terms (each
    split hi/lo in bf16 to keep f32-level accuracy), so PSUM directly
    holds X = -d^2/2.
  - ScalarE evaluates d = exp(0.5*ln(-2X)) (Ln+Exp share one ACT table
    set; Sqrt would force table thrashing and has a loose ULP budget),
    then exp(-10*d + b_i) with per-row bias b_i = 10*d_ii - 40 and a
    fused free-dim accumulation (accum_out) giving the softmax sums S_i.
    Algebra: logsumexp_i - logit_ii == ln(S_i) + 40 exactly, so only
    S_i [B] leaves the device for the contrastive term.
  - The magnitude-loss numerator sum_d |p - t| is reduced over the
    contraction dim with a ones-vector matmul on PE.
  - Host does the O(B*D) input prep (transpose/shard/row stats) and the
    final O(B) reduction of the per-row scalars; all O(B^2 D) and
    O(B^2) work runs on the NeuronCores.

Outputs per core: S [128,4] f32, l1 [1,512] f32  ->  host combines to
(total_loss, contrastive_dist_loss, normalized_magnitude_loss).
"""

import numpy as np
import ml_dtypes

BF16 = ml_dtypes.bfloat16

B = 4096
D = 768
NCORES = 8
BL = B // NCORES          # 512 rows per core
P = 128                   # partitions
NK = D // P               # 6 full contraction chunks
KEXT = 4                  # hi/lo tsq + hi/lo psq rank-1 rows
NT = BL // P              # 4 m-tiles per core
NJ = B // 512             # 8 n-chunks of 512
C_STAB = 40.0             # stabilization constant; see module docstring

_COMPILED = None          # cached (nc) bass program
LAST_RESULTS = None       # BassKernelResults of the most recent run


def _build_bass():
    import concourse.bass as bass
    import concourse.mybir as mybir
    import concourse.tile as tile
    from concourse import bacc
    from contextlib import ExitStack

    f32 = mybir.dt.float32
    bf16 = mybir.dt.bfloat16

    nc = bacc.Bacc("TRN2", target_bir_lowering=False, debug=False,
                   num_devices=NCORES)

    pt_d = nc.dram_tensor("pt_ext", [D + KEXT, BL], bf16,
                          kind="ExternalInput").ap()
    tt_d = nc.dram_tensor("tt_ext", [D + KEXT, B], bf16,
                          kind="ExternalInput").ap()
    ts_d = nc.dram_tensor("ts_ext", [D, BL], bf16,
                          kind="ExternalInput").ap()
    bias_d = nc.dram_tensor("bias_in", [P, NT], f32,
                            kind="ExternalInput").ap()
    s_d = nc.dram_tensor("s_out", [P, NT], f32,
                         kind="ExternalOutput").ap()
    l1_d = nc.dram_tensor("l1_out", [1, BL], f32,
                          kind="ExternalOutput").ap()

    with tile.TileContext(nc) as tc, ExitStack() as ctx:
        const_pool = ctx.enter_context(tc.tile_pool(name="consts", bufs=1))
        work_pool = ctx.enter_context(tc.tile_pool(name="work", bufs=2))
        big_pool = ctx.enter_context(tc.tile_pool(name="big", bufs=2))
        psum_x = ctx.enter_context(
            tc.tile_pool(name="psum_x", bufs=7, space="PSUM"))
        psum_l1 = ctx.enter_context(
            tc.tile_pool(name="psum_l1", bufs=1, space="PSUM"))

        # ---- input loads (resident for the whole kernel) ----
        tt_sb = []
        pt_sb = []
        ts_sb = []
        for k in range(NK):
            ttk = const_pool.tile([P, B], bf16, name=f"tt{k}")
            nc.sync.dma_start(ttk, tt_d[k * P:(k + 1) * P, :])
            tt_sb.append(ttk)
            ptk = const_pool.tile([P, BL], bf16, name=f"pt{k}")
            nc.sync.dma_start(ptk, pt_d[k * P:(k + 1) * P, :])
            pt_sb.append(ptk)
            tsk = const_pool.tile([P, BL], bf16, name=f"ts{k}")
            nc.sync.dma_start(tsk, ts_d[k * P:(k + 1) * P, :])
            ts_sb.append(tsk)
        tt6 = const_pool.tile([KEXT, B], bf16, name="tt6")
        nc.sync.dma_start(tt6, tt_d[D:D + KEXT, :])
        tt_sb.append(tt6)
        pt6 = const_pool.tile([KEXT, BL], bf16, name="pt6")
        nc.sync.dma_start(pt6, pt_d[D:D + KEXT, :])
        pt_sb.append(pt6)

        bias_sb = const_pool.tile([P, NT], f32, name="bias_sb")
        nc.sync.dma_start(bias_sb, bias_d)
        ones_sb = const_pool.tile([P, 1], bf16, name="ones_sb")
        nc.gpsimd.memset(ones_sb, 1.0)

        s_sb = const_pool.tile([P, NT], f32, name="s_sb")
        l1_sb = const_pool.tile([1, BL], f32, name="l1_sb")

        # ---- magnitude-loss numerator: l1[m] = sum_d |p - t| ----
        # (also serves as PE warmup before the big matmul stream)
        l1_ps = psum_l1.tile([1, BL], f32, name="l1_ps")
        for k in range(NK):
            diff = work_pool.tile([P, BL], bf16, name="diff", tag="diff")
            nc.vector.tensor_tensor(diff, pt_sb[k], ts_sb[k],
                                    op=mybir.AluOpType.subtract)
            ndiff = work_pool.tile([P, BL], bf16, name="ndiff", tag="ndiff")
            nc.vector.tensor_scalar(ndiff, diff, -1.0, None,
                                    op0=mybir.AluOpType.mult)
            absd = work_pool.tile([P, BL], bf16, name="absd", tag="absd")
            nc.vector.tensor_tensor(absd, diff, ndiff,
                                    op=mybir.AluOpType.max)
            nc.tensor.matmul(l1_ps, lhsT=ones_sb, rhs=absd,
                             start=(k == 0), stop=(k == NK - 1))
        nc.vector.tensor_copy(l1_sb, l1_ps)
        nc.sync.dma_start(l1_d, l1_sb)

        # ---- main: X = -d^2/2 on PE; d = exp(.5 ln(-2X)); softmax sums ----
        for t in range(NT):
            lnq = big_pool.tile([P, B], f32, name="lnq", tag="lnq")
            for j in range(NJ):
                x_ps = psum_x.tile([P, 512], f32, name="x_ps", tag="x")
                for k in range(NK + 1):
                    nc.tensor.matmul(
                        x_ps,
                        lhsT=pt_sb[k][:, t * P:(t + 1) * P],
                        rhs=tt_sb[k][:, j * 512:(j + 1) * 512],
                        start=(k == 0), stop=(k == NK))
                nc.scalar.activation(lnq[:, j * 512:(j + 1) * 512], x_ps,
                                     mybir.ActivationFunctionType.Ln,
                                     scale=-2.0)
            dmat = big_pool.tile([P, B], f32, name="dmat", tag="dmat")
            nc.scalar.activation(dmat, lnq,
                                 mybir.ActivationFunctionType.Exp,
                                 scale=0.5)
            emat = big_pool.tile([P, B], f32, name="emat", tag="emat")
            nc.scalar.activation(emat, dmat,
                                 mybir.ActivationFunctionType.Exp,
                                 scale=-10.0,
                                 bias=bias_sb[:, t:t + 1],
                                 accum_out=s_sb[:, t:t + 1])
        nc.sync.dma_start(s_d, s_sb)

    nc.compile()
    return nc


def _get_compiled():
    global _COMPILED
    if _COMPILED is None:
        _COMPILED = _build_bass()
    return _COMPILED


def _split_bf16(v):
    hi = v.astype(np.float32).astype(BF16)
    lo = (v.astype(np.float32) - hi.astype(np.float32)).astype(BF16)
    return hi, lo


def kernel(predicted, target):
    global LAST_RESULTS
    from concourse.bass_utils import run_bass_kernel_spmd

    p = np.ascontiguousarray(np.asarray(predicted, dtype=np.float32))
    t = np.ascontiguousarray(np.asarray(target, dtype=np.float32))
    assert p.shape == (B, D) and t.shape == (B, D)

    # host-side O(B*D) row stats (input prep for the device program)
    p64 = p.astype(np.float64)
    t64 = t.astype(np.float64)
    psq = (p64 * p64).sum(1)
    tsq = (t64 * t64).sum(1)
    tmag = np.abs(t64).sum(1)
    dii = np.sqrt(((p64 - t64) ** 2).sum(1))

    tt_ext = np.zeros((D + KEXT, B), dtype=BF16)
    tt_ext[:D] = np.ascontiguousarray(t.T).astype(BF16)
    hi, lo = _split_bf16(-0.5 * tsq)
    tt_ext[D + 0] = hi
    tt_ext[D + 1] = lo
    tt_ext[D + 2] = BF16(1.0)
    tt_ext[D + 3] = BF16(1.0)

    in_maps = []
    for c in range(NCORES):
        sl = slice(c * BL, (c + 1) * BL)
        pt_ext = np.zeros((D + KEXT, BL), dtype=BF16)
        pt_ext[:D] = np.ascontiguousarray(p[sl].T).astype(BF16)
        pt_ext[D + 0] = BF16(1.0)
        pt_ext[D + 1] = BF16(1.0)
        hi, lo = _split_bf16(-0.5 * psq[sl])
        pt_ext[D + 2] = hi
        pt_ext[D + 3] = lo
        ts_ext = np.ascontiguousarray(t[sl].T).astype(BF16)
        bias = np.ascontiguousarray(
            (10.0 * dii[sl] - C_STAB).astype(np.float32).reshape(NT, P).T)
        in_maps.append({
            "pt_ext": pt_ext,
            "tt_ext": tt_ext,
            "ts_ext": ts_ext,
            "bias_in": bias,
        })

    nc = _get_compiled()
    res = run_bass_kernel_spmd(nc, in_maps, core_ids=list(range(NCORES)))
    LAST_RESULTS = res

    S = np.empty(B, dtype=np.float64)
    l1 = np.empty(B, dtype=np.float64)
    for c in range(NCORES):
        out = res.results[c]
        S[c * BL:(c + 1) * BL] = out["s_out"].T.reshape(BL)
        l1[c * BL:(c + 1) * BL] = out["l1_out"].reshape(BL)

    contrastive = float(np.log(S).mean() + C_STAB)
    magnitude = float((l1 / tmag).mean())
    total = 0.5 * contrastive + 0.5 * magnitude
    return (np.float32(total), np.float32(contrastive), np.float32(magnitude))


# revision 7
# speedup vs baseline: 1.0210x; 1.0210x over previous
"""ContrastiveMagnitudeLoss on 8 Trainium2 NeuronCores (Bass/Tile).

Strategy (sharding_hint: shard batch across cores, all-gather target):
  - B=4096 rows of `predicted` are sharded 512/core. Every core gets the
    full (transposed) `target`, so each core owns complete rows of the
    B x B distance matrix and the row-softmax needs no communication.
  - The Gram identity  d^2[m,n] = ||p_m||^2 + ||t_n||^2 - 2 p_m.t_n  is
    computed entirely on the PE array by extending the contraction dim:
    4 extra K-rows carry (1, -tsq/2) and (-psq/2, 1) rank-1 terms (each
    split hi/lo in bf16 to keep f32-level accuracy), so PSUM directly
    holds X = -d^2/2.
  - ScalarE evaluates d = exp(0.5*ln(-2X)) (Ln+Exp share one ACT table
    set; Sqrt would force table thrashing and has a loose ULP budget),
    then exp(-10*d + b_i) with per-row bias b_i = 10*d_ii - 40 and a
    fused free-dim accumulation (accum_out) giving the softmax sums S_i.
    Algebra: logsumexp_i - logit_ii == ln(S_i) + 40 exactly, so only
    S_i [B] leaves the device for the contrastive term.
  - The magnitude-loss numerator sum_d |p - t| is reduced over the
    contraction dim with a ones-vector matmul on PE.
  - Host does the O(B*D) input prep (transpose/shard/row stats) and the
    final O(B) reduction of the per-row scalars; all O(B^2 D) and
    O(B^2) work runs on the NeuronCores.

Outputs per core: S [128,4] f32, l1 [1,512] f32  ->  host combines to
(total_loss, contrastive_dist_loss, normalized_magnitude_loss).
"""

import numpy as np
import ml_dtypes

BF16 = ml_dtypes.bfloat16

B = 4096
D = 768
NCORES = 8
BL = B // NCORES          # 512 rows per core
P = 128                   # partitions
NK = D // P               # 6 full contraction chunks
KEXT = 4                  # hi/lo tsq + hi/lo psq rank-1 rows
NT = BL // P              # 4 m-tiles per core
NJ = B // 512             # 8 n-chunks of 512
C_STAB = 40.0             # stabilization constant; see module docstring

_COMPILED = None          # cached (nc) bass program
LAST_RESULTS = None       # BassKernelResults of the most recent run


def _build_bass():
    import concourse.bass as bass
    import concourse.mybir as mybir
    import concourse.tile as tile
    import concourse.hw_specs as hw_specs
    from concourse import bacc
    from contextlib import ExitStack

    f32 = mybir.dt.float32
    bf16 = mybir.dt.bfloat16

    # Both Ln and Exp live in the 'natural_log_exp_and_others' ACT table
    # set, but the table-load placement pass resolves each function to the
    # first set containing it (exp_and_others / natural_log), which makes
    # interleaved Ln/Exp reload tables ~14x (~2.7us each). Present those
    # two single-function sets as empty (indices preserved) so both
    # functions resolve to the combined set -> exactly one table load.
    orig_tables = hw_specs.get_activation_tables

    def _tables_one_set(arch):
        t = dict(orig_tables(arch))
        t["exp_and_others"] = set()
        t["natural_log"] = set()
        return t

    hw_specs.get_activation_tables = _tables_one_set
    bacc.get_activation_tables = _tables_one_set
    try:
        return _build_bass_inner(nc_cls=bacc.Bacc)
    finally:
        hw_specs.get_activation_tables = orig_tables
        bacc.get_activation_tables = orig_tables


def _build_bass_inner(nc_cls):
    import concourse.mybir as mybir
    import concourse.tile as tile
    from contextlib import ExitStack

    f32 = mybir.dt.float32
    bf16 = mybir.dt.bfloat16

    nc = nc_cls("TRN2", target_bir_lowering=False, debug=False,
                num_devices=NCORES)

    pt_d = nc.dram_tensor("pt_ext", [D + KEXT, BL], bf16,
                          kind="ExternalInput").ap()
    tt_d = nc.dram_tensor("tt_ext", [D + KEXT, B], bf16,
                          kind="ExternalInput").ap()
    ts_d = nc.dram_tensor("ts_ext", [D, BL], bf16,
                          kind="ExternalInput").ap()
    bias_d = nc.dram_tensor("bias_in", [P, NT], f32,
                            kind="ExternalInput").ap()
    s_d = nc.dram_tensor("s_out", [P, NT], f32,
                         kind="ExternalOutput").ap()
    l1_d = nc.dram_tensor("l1_out", [1, BL], f32,
                          kind="ExternalOutput").ap()

    with tile.TileContext(nc) as tc, ExitStack() as ctx:
        const_pool = ctx.enter_context(tc.tile_pool(name="consts", bufs=1))
        work_pool = ctx.enter_context(tc.tile_pool(name="work", bufs=2))
        big_pool = ctx.enter_context(tc.tile_pool(name="big", bufs=2))

        # ---- input loads (resident for the whole kernel) ----
        tt_sb = []
        pt_sb = []
        ts_sb = []
        for k in range(NK):
            ttk = const_pool.tile([P, B], bf16, name=f"tt{k}")
            nc.sync.dma_start(ttk, tt_d[k * P:(k + 1) * P, :])
            tt_sb.append(ttk)
            ptk = const_pool.tile([P, BL], bf16, name=f"pt{k}")
            nc.sync.dma_start(ptk, pt_d[k * P:(k + 1) * P, :])
            pt_sb.append(ptk)
            tsk = const_pool.tile([P, BL], bf16, name=f"ts{k}")
            nc.sync.dma_start(tsk, ts_d[k * P:(k + 1) * P, :])
            ts_sb.append(tsk)
        tt6 = const_pool.tile([KEXT, B], bf16, name="tt6")
        nc.sync.dma_start(tt6, tt_d[D:D + KEXT, :])
        tt_sb.append(tt6)
        pt6 = const_pool.tile([KEXT, BL], bf16, name="pt6")
        nc.sync.dma_start(pt6, pt_d[D:D + KEXT, :])
        pt_sb.append(pt6)

        bias_sb = const_pool.tile([P, NT], f32, name="bias_sb")
        nc.sync.dma_start(bias_sb, bias_d)
        ones_sb = const_pool.tile([P, 1], bf16, name="ones_sb")
        nc.gpsimd.memset(ones_sb, 1.0)

        s_sb = const_pool.tile([P, NT], f32, name="s_sb")
        l1_sb = const_pool.tile([1, BL], f32, name="l1_sb")

        # ---- magnitude-loss numerator: l1[m] = sum_d |p - t| ----
        # (also serves as PE warmup before the big matmul stream; its PSUM
        # pool closes before the main pool opens so the 8 banks are free)
        with tc.tile_pool(name="psum_l1", bufs=1, space="PSUM") as psum_l1:
            l1_ps = psum_l1.tile([1, BL], f32, name="l1_ps")
            for k in range(NK):
                diff = work_pool.tile([P, BL], bf16, name="diff", tag="diff")
                nc.vector.tensor_tensor(diff, pt_sb[k], ts_sb[k],
                                        op=mybir.AluOpType.subtract)
                ndiff = work_pool.tile([P, BL], bf16, name="ndiff",
                                       tag="ndiff")
                nc.vector.tensor_scalar(ndiff, diff, -1.0, None,
                                        op0=mybir.AluOpType.mult)
                absd = work_pool.tile([P, BL], bf16, name="absd", tag="absd")
                nc.vector.tensor_tensor(absd, diff, ndiff,
                                        op=mybir.AluOpType.max)
                nc.tensor.matmul(l1_ps, lhsT=ones_sb, rhs=absd,
                                 start=(k == 0), stop=(k == NK - 1))
            nc.vector.tensor_copy(l1_sb, l1_ps)
            nc.sync.dma_start(l1_d, l1_sb)

        # ---- main: X = -d^2/2 on PE; d = exp(.5 ln(-2X)); softmax sums ----
        # k-outer / j-inner: 8 consecutive matmuls share one stationary
        # operand, so the weight load amortizes and matmuls stream
        # back-to-back at ~N cycles.
        with tc.tile_pool(name="psum_x", bufs=8, space="PSUM") as psum_x:
            for t in range(NT):
                lnq = big_pool.tile([P, B], f32, name="lnq", tag="lnq")
                x_ps = [psum_x.tile([P, 512], f32, name=f"x_ps{j}", tag="x")
                        for j in range(NJ)]
                for k in range(NK + 1):
                    for j in range(NJ):
                        nc.tensor.matmul(
                            x_ps[j],
                            lhsT=pt_sb[k][:, t * P:(t + 1) * P],
                            rhs=tt_sb[k][:, j * 512:(j + 1) * 512],
                            start=(k == 0), stop=(k == NK))
                for j in range(NJ):
                    nc.scalar.activation(lnq[:, j * 512:(j + 1) * 512],
                                         x_ps[j],
                                         mybir.ActivationFunctionType.Ln,
                                         scale=-2.0)
                dmat = big_pool.tile([P, B], f32, name="dmat", tag="dmat")
                nc.scalar.activation(dmat, lnq,
                                     mybir.ActivationFunctionType.Exp,
                                     scale=0.5)
                emat = big_pool.tile([P, B], f32, name="emat", tag="emat")
                nc.scalar.activation(emat, dmat,
                                     mybir.ActivationFunctionType.Exp,
                                     scale=-10.0,
                                     bias=bias_sb[:, t:t + 1],
                                     accum_out=s_sb[:, t:t + 1])
            nc.sync.dma_start(s_d, s_sb)

    nc.compile()
    return nc


def _get_compiled():
    global _COMPILED
    if _COMPILED is None:
        _COMPILED = _build_bass()
    return _COMPILED


def _split_bf16(v):
    hi = v.astype(np.float32).astype(BF16)
    lo = (v.astype(np.float32) - hi.astype(np.float32)).astype(BF16)
    return hi, lo


def kernel(predicted, target):
    global LAST_RESULTS
    from concourse.bass_utils import run_bass_kernel_spmd

    p = np.ascontiguousarray(np.asarray(predicted, dtype=np.float32))
    t = np.ascontiguousarray(np.asarray(target, dtype=np.float32))
    assert p.shape == (B, D) and t.shape == (B, D)

    # host-side O(B*D) row stats (input prep for the device program)
    p64 = p.astype(np.float64)
    t64 = t.astype(np.float64)
    psq = (p64 * p64).sum(1)
    tsq = (t64 * t64).sum(1)
    tmag = np.abs(t64).sum(1)
    dii = np.sqrt(((p64 - t64) ** 2).sum(1))

    tt_ext = np.zeros((D + KEXT, B), dtype=BF16)
    tt_ext[:D] = np.ascontiguousarray(t.T).astype(BF16)
    hi, lo = _split_bf16(-0.5 * tsq)
    tt_ext[D + 0] = hi
    tt_ext[D + 1] = lo
    tt_ext[D + 2] = BF16(1.0)
    tt_ext[D + 3] = BF16(1.0)

    in_maps = []
    for c in range(NCORES):
        sl = slice(c * BL, (c + 1) * BL)
        pt_ext = np.zeros((D + KEXT, BL), dtype=BF16)
        pt_ext[:D] = np.ascontiguousarray(p[sl].T).astype(BF16)
        pt_ext[D + 0] = BF16(1.0)
        pt_ext[D + 1] = BF16(1.0)
        hi, lo = _split_bf16(-0.5 * psq[sl])
        pt_ext[D + 2] = hi
        pt_ext[D + 3] = lo
        ts_ext = np.ascontiguousarray(t[sl].T).astype(BF16)
        bias = np.ascontiguousarray(
            (10.0 * dii[sl] - C_STAB).astype(np.float32).reshape(NT, P).T)
        in_maps.append({
            "pt_ext": pt_ext,
            "tt_ext": tt_ext,
            "ts_ext": ts_ext,
            "bias_in": bias,
        })

    nc = _get_compiled()
    res = run_bass_kernel_spmd(nc, in_maps, core_ids=list(range(NCORES)))
    LAST_RESULTS = res

    S = np.empty(B, dtype=np.float64)
    l1 = np.empty(B, dtype=np.float64)
    for c in range(NCORES):
        out = res.results[c]
        S[c * BL:(c + 1) * BL] = out["s_out"].T.reshape(BL)
        l1[c * BL:(c + 1) * BL] = out["l1_out"].reshape(BL)

    contrastive = float(np.log(S).mean() + C_STAB)
    magnitude = float((l1 / tmag).mean())
    total = 0.5 * contrastive + 0.5 * magnitude
    return (np.float32(total), np.float32(contrastive), np.float32(magnitude))


# revision 10
# speedup vs baseline: 1.1919x; 1.1674x over previous
"""ContrastiveMagnitudeLoss on 8 Trainium2 NeuronCores (Bass/Tile).

Strategy (sharding_hint: shard batch across cores, all-gather target):
  - B=4096 rows of `predicted` are sharded 512/core. Every core gets the
    full (transposed) `target`, so each core owns complete rows of the
    B x B distance matrix and the row-softmax needs no communication.
  - The Gram identity  d^2[m,n] = ||p_m||^2 + ||t_n||^2 - 2 p_m.t_n  is
    computed entirely on the PE array by extending the contraction dim:
    4 extra K-rows carry (1, -tsq/2) and (-psq/2, 1) rank-1 terms (each
    split hi/lo in bf16 to keep f32-level accuracy), so PSUM directly
    holds X = -d^2/2.
  - ScalarE evaluates d = exp(0.5*ln(-2X)) (Ln+Exp share one ACT table
    set; Sqrt would force table thrashing and has a loose ULP budget),
    then exp(-10*d + b_i) with per-row bias b_i = 10*d_ii - 40 and a
    fused free-dim accumulation (accum_out) giving the softmax sums S_i.
    Algebra: logsumexp_i - logit_ii == ln(S_i) + 40 exactly, so only
    S_i [B] leaves the device for the contrastive term.
  - The magnitude-loss numerator sum_d |p - t| is reduced over the
    contraction dim with a ones-vector matmul on PE.
  - Host does the O(B*D) input prep (transpose/shard/row stats) and the
    final O(B) reduction of the per-row scalars; all O(B^2 D) and
    O(B^2) work runs on the NeuronCores.

Outputs per core: S [128,4] f32, l1 [1,512] f32  ->  host combines to
(total_loss, contrastive_dist_loss, normalized_magnitude_loss).
"""

import numpy as np
import ml_dtypes

BF16 = ml_dtypes.bfloat16

B = 4096
D = 768
NCORES = 8
BL = B // NCORES          # 512 rows per core
P = 128                   # partitions
NK = D // P               # 6 full contraction chunks
KEXT = 4                  # hi/lo tsq + hi/lo psq rank-1 rows
NT = BL // P              # 4 m-tiles per core
NJ = B // 512             # 8 n-chunks of 512
C_STAB = 40.0             # stabilization constant; see module docstring

_COMPILED = None          # cached (nc) bass program
LAST_RESULTS = None       # BassKernelResults of the most recent run


def _build_bass():
    import concourse.bass as bass
    import concourse.mybir as mybir
    import concourse.tile as tile
    import concourse.hw_specs as hw_specs
    from concourse import bacc
    from contextlib import ExitStack

    f32 = mybir.dt.float32
    bf16 = mybir.dt.bfloat16

    # Both Ln and Exp live in the 'natural_log_exp_and_others' ACT table
    # set, but the table-load placement pass resolves each function to the
    # first set containing it (exp_and_others / natural_log), which makes
    # interleaved Ln/Exp reload tables ~14x (~2.7us each). Present those
    # two single-function sets as empty (indices preserved) so both
    # functions resolve to the combined set -> exactly one table load.
    orig_tables = hw_specs.get_activation_tables

    def _tables_one_set(arch):
        t = dict(orig_tables(arch))
        t["exp_and_others"] = set()
        t["natural_log"] = set()
        return t

    hw_specs.get_activation_tables = _tables_one_set
    bacc.get_activation_tables = _tables_one_set
    try:
        return _build_bass_inner(nc_cls=bacc.Bacc)
    finally:
        hw_specs.get_activation_tables = orig_tables
        bacc.get_activation_tables = orig_tables


def _build_bass_inner(nc_cls):
    import concourse.mybir as mybir
    import concourse.tile as tile
    from contextlib import ExitStack

    f32 = mybir.dt.float32
    bf16 = mybir.dt.bfloat16

    nc = nc_cls("TRN2", target_bir_lowering=False, debug=False,
                num_devices=NCORES)

    pt_d = nc.dram_tensor("pt_ext", [D + KEXT, BL], bf16,
                          kind="ExternalInput").ap()
    tt_d = nc.dram_tensor("tt_ext", [D + KEXT, B], bf16,
                          kind="ExternalInput").ap()
    ts_d = nc.dram_tensor("ts_ext", [D, BL], bf16,
                          kind="ExternalInput").ap()
    bias_d = nc.dram_tensor("bias_in", [P, NT], f32,
                            kind="ExternalInput").ap()
    s_d = nc.dram_tensor("s_out", [P, 2 * NT], f32,
                         kind="ExternalOutput").ap()
    l1_d = nc.dram_tensor("l1_out", [1, BL], f32,
                          kind="ExternalOutput").ap()

    with tile.TileContext(nc) as tc, ExitStack() as ctx:
        const_pool = ctx.enter_context(tc.tile_pool(name="consts", bufs=1))
        work_pool = ctx.enter_context(tc.tile_pool(name="work", bufs=2))
        big_pool = ctx.enter_context(tc.tile_pool(name="big", bufs=2))

        HB = B // 2           # 2048: column half processed per ACT step

        # ---- input loads ----
        # Small per-core tensors go on the GpSimd DMA queue; the 6.3 MB
        # tt_ext streams on the Sync queue split into column halves so the
        # PE can start the first half-sweep after ~1/7 of the stream.
        bias_sb = const_pool.tile([P, NT], f32, name="bias_sb")
        nc.gpsimd.dma_start(bias_sb, bias_d)
        pt_sb = []
        for k in range(NK):
            ptk = const_pool.tile([P, BL], bf16, name=f"pt{k}")
            nc.gpsimd.dma_start(ptk, pt_d[k * P:(k + 1) * P, :])
            pt_sb.append(ptk)
        pt6 = const_pool.tile([KEXT, BL], bf16, name="pt6")
        nc.gpsimd.dma_start(pt6, pt_d[D:D + KEXT, :])
        pt_sb.append(pt6)

        tt_sb = [const_pool.tile([P, B], bf16, name=f"tt{k}")
                 for k in range(NK)]
        tt_sb.append(const_pool.tile([KEXT, B], bf16, name="tt6"))
        for h in range(2):
            cols = slice(h * HB, (h + 1) * HB)
            for k in range(NK):
                nc.sync.dma_start(tt_sb[k][:, cols],
                                  tt_d[k * P:(k + 1) * P, cols])
            nc.sync.dma_start(tt_sb[NK][:, cols], tt_d[D:D + KEXT, cols])

        ts_sb = []
        for k in range(NK):
            tsk = const_pool.tile([P, BL], bf16, name=f"ts{k}")
            nc.gpsimd.dma_start(tsk, ts_d[k * P:(k + 1) * P, :])
            ts_sb.append(tsk)

        ones_sb = const_pool.tile([P, 1], bf16, name="ones_sb")
        nc.gpsimd.memset(ones_sb, 1.0)

        s_sb = const_pool.tile([P, 2 * NT], f32, name="s_sb")
        l1_sb = const_pool.tile([1, BL], f32, name="l1_sb")

        # ---- main: X = -d^2/2 on PE; d = exp(.5 ln(-2X)); softmax sums ----
        # Per m-tile and column half: a k-outer sweep (runs of 4 matmuls
        # share one stationary operand) accumulates a 4-bank PSUM quad,
        # one Ln drains it to SBUF, then exp(.5*)/exp(-10*+b) with fused
        # row-accumulation produce the softmax partial sums.
        with tc.tile_pool(name="psum_x", bufs=2, space="PSUM") as psum_x:
            for t in range(NT):
                lnq = big_pool.tile([P, B], f32, name="lnq", tag="lnq")
                for h in range(2):
                    xq = psum_x.tile([P, HB], f32, name="xq", tag="xq")
                    for k in range(NK + 1):
                        for jl in range(4):
                            nc.tensor.matmul(
                                xq[:, jl * 512:(jl + 1) * 512],
                                lhsT=pt_sb[k][:, t * P:(t + 1) * P],
                                rhs=tt_sb[k][:, h * HB + jl * 512:
                                             h * HB + (jl + 1) * 512],
                                start=(k == 0), stop=(k == NK))
                    hcols = slice(h * HB, (h + 1) * HB)
                    nc.scalar.activation(lnq[:, hcols], xq,
                                         mybir.ActivationFunctionType.Ln,
                                         scale=-2.0)
                    dmat = big_pool.tile([P, HB], f32, name="dmat",
                                         tag="dmat")
                    nc.scalar.activation(dmat, lnq[:, hcols],
                                         mybir.ActivationFunctionType.Exp,
                                         scale=0.5)
                    emat = big_pool.tile([P, HB], f32, name="emat",
                                         tag="emat")
                    nc.scalar.activation(emat, dmat,
                                         mybir.ActivationFunctionType.Exp,
                                         scale=-10.0,
                                         bias=bias_sb[:, t:t + 1],
                                         accum_out=s_sb[:, 2 * t + h:
                                                        2 * t + h + 1])
            nc.sync.dma_start(s_d, s_sb)

        # ---- magnitude-loss numerator: l1[m] = sum_d |p - t| ----
        # Runs after the main loop: its DVE prep overlaps the matmul body
        # and its 6 small matmuls fill the PE idle tail while ScalarE
        # finishes the last exponentials.
        with tc.tile_pool(name="psum_l1", bufs=1, space="PSUM") as psum_l1:
            l1_ps = psum_l1.tile([1, BL], f32, name="l1_ps")
            for k in range(NK):
                diff = work_pool.tile([P, BL], bf16, name="diff", tag="diff")
                nc.vector.tensor_tensor(diff, pt_sb[k], ts_sb[k],
                                        op=mybir.AluOpType.subtract)
                ndiff = work_pool.tile([P, BL], bf16, name="ndiff",
                                       tag="ndiff")
                nc.vector.tensor_scalar(ndiff, diff, -1.0, None,
                                        op0=mybir.AluOpType.mult)
                absd = work_pool.tile([P, BL], bf16, name="absd", tag="absd")
                nc.vector.tensor_tensor(absd, diff, ndiff,
                                        op=mybir.AluOpType.max)
                nc.tensor.matmul(l1_ps, lhsT=ones_sb, rhs=absd,
                                 start=(k == 0), stop=(k == NK - 1))
            nc.vector.tensor_copy(l1_sb, l1_ps)
            nc.sync.dma_start(l1_d, l1_sb)

    nc.compile()
    return nc


def _get_compiled():
    global _COMPILED
    if _COMPILED is None:
        _COMPILED = _build_bass()
    return _COMPILED


def _split_bf16(v):
    hi = v.astype(np.float32).astype(BF16)
    lo = (v.astype(np.float32) - hi.astype(np.float32)).astype(BF16)
    return hi, lo


def kernel(predicted, target):
    global LAST_RESULTS
    from concourse.bass_utils import run_bass_kernel_spmd

    p = np.ascontiguousarray(np.asarray(predicted, dtype=np.float32))
    t = np.ascontiguousarray(np.asarray(target, dtype=np.float32))
    assert p.shape == (B, D) and t.shape == (B, D)

    # host-side O(B*D) row stats (input prep for the device program)
    p64 = p.astype(np.float64)
    t64 = t.astype(np.float64)
    psq = (p64 * p64).sum(1)
    tsq = (t64 * t64).sum(1)
    tmag = np.abs(t64).sum(1)
    dii = np.sqrt(((p64 - t64) ** 2).sum(1))

    tt_ext = np.zeros((D + KEXT, B), dtype=BF16)
    tt_ext[:D] = np.ascontiguousarray(t.T).astype(BF16)
    hi, lo = _split_bf16(-0.5 * tsq)
    tt_ext[D + 0] = hi
    tt_ext[D + 1] = lo
    tt_ext[D + 2] = BF16(1.0)
    tt_ext[D + 3] = BF16(1.0)

    in_maps = []
    for c in range(NCORES):
        sl = slice(c * BL, (c + 1) * BL)
        pt_ext = np.zeros((D + KEXT, BL), dtype=BF16)
        pt_ext[:D] = np.ascontiguousarray(p[sl].T).astype(BF16)
        pt_ext[D + 0] = BF16(1.0)
        pt_ext[D + 1] = BF16(1.0)
        hi, lo = _split_bf16(-0.5 * psq[sl])
        pt_ext[D + 2] = hi
        pt_ext[D + 3] = lo
        ts_ext = np.ascontiguousarray(t[sl].T).astype(BF16)
        bias = np.ascontiguousarray(
            (10.0 * dii[sl] - C_STAB).astype(np.float32).reshape(NT, P).T)
        in_maps.append({
            "pt_ext": pt_ext,
            "tt_ext": tt_ext,
            "ts_ext": ts_ext,
            "bias_in": bias,
        })

    nc = _get_compiled()
    res = run_bass_kernel_spmd(nc, in_maps, core_ids=list(range(NCORES)))
    LAST_RESULTS = res

    S = np.empty(B, dtype=np.float64)
    l1 = np.empty(B, dtype=np.float64)
    for c in range(NCORES):
        out = res.results[c]
        # s_out[p, 2t+h] = softmax partial over column half h for row
        # c*BL + t*P + p; the two halves sum to the full row sum.
        s = out["s_out"].astype(np.float64)
        s_full = s[:, 0::2] + s[:, 1::2]            # [P, NT]
        S[c * BL:(c + 1) * BL] = s_full.T.reshape(BL)
        l1[c * BL:(c + 1) * BL] = out["l1_out"].reshape(BL)

    contrastive = float(np.log(S).mean() + C_STAB)
    magnitude = float((l1 / tmag).mean())
    total = 0.5 * contrastive + 0.5 * magnitude
    return (np.float32(total), np.float32(contrastive), np.float32(magnitude))
